# revision 29
# baseline (speedup 1.0000x reference)
"""Trainium2 Bass kernel for a BasicTransformerBlock (self-attn + cross-attn +
GEGLU FFN), sharded over 8 NeuronCores.

Sharding: data-parallel over batch (2) x sequence chunks (4): core c handles
batch c//4, query rows [(c%4)*1024, (c%4+1)*1024). Each core recomputes
LN1+K/V over its batch's full 4096-row sequence (needed for self-attention)
and produces its own 1024-row output chunk. No collectives.

Layout convention on device: residual stream is token-major f32 [128_tok, 512]
tiles; matmul operands are bf16; activations are transposed to feature-major
[feat_part, tok_free] with PE transposes so the tensor engine can contract
over features; attention probabilities stay feature-major [key_part, query]
so exp'd tiles feed attn@v directly as the stationary operand.
"""

import numpy as np
import ml_dtypes
from contextlib import ExitStack

import concourse.bass as bass
import concourse.tile as tile
from concourse import bacc, mybir
from concourse import bass_utils

F32 = mybir.dt.float32
BF16 = mybir.dt.bfloat16
F8 = mybir.dt.float8e4
AF = mybir.ActivationFunctionType
ALU = mybir.AluOpType
DR = mybir.MatmulPerfMode.DoubleRow

# problem constants (hardcoded per the harness contract)
B = 2
N = 4096          # self-attn sequence length (per batch)
NC = 1024         # per-core query chunk
D = 512           # model dim
H = 8             # heads
DH = 64           # head dim
M = 256           # context length
CD = 768          # context dim
FF = 2048         # GEGLU inner dim
LN_EPS = 1e-5
SCALE = DH ** -0.5

NT = N // 128      # 32 x_full tiles
NCT = NC // 128    # 8 own tiles


def _ln_group(nc, stat_pool, h_pool, tiles, name=""):
    """LayerNorm a group of token-major f32 tiles [128,512] -> bf16 list.
    Stats are batched: one reciprocal + one sqrt instruction per group."""
    n = len(tiles)
    aggrs = []
    veps = stat_pool.tile([128, n], F32, name=f"ve{name}", tag="ve")
    for t, xt in enumerate(tiles):
        stats = stat_pool.tile([128, 6], F32, name=f"st{name}", tag="st")
        nc.vector.bn_stats(stats[:], xt[:])
        aggr = stat_pool.tile([128, 2], F32, name=f"ag{name}", tag="ag")
        nc.vector.bn_aggr(aggr[:], stats[:])
        nc.vector.tensor_scalar_add(veps[:, t:t + 1], aggr[:, 1:2], LN_EPS)
        aggrs.append(aggr)
    rvar = stat_pool.tile([128, n], F32, name=f"rv{name}", tag="rv")
    nc.vector.reciprocal(rvar[:], veps[:])
    rstd = stat_pool.tile([128, n], F32, name=f"rs{name}", tag="rs")
    nc.scalar.sqrt(rstd[:], rvar[:])
    hs = []
    for t, xt in enumerate(tiles):
        h = h_pool.tile([128, D], BF16, name=f"h{name}", tag="h")
        nc.vector.tensor_scalar(h[:], xt[:], aggrs[t][:, 0:1],
                                rstd[:, t:t + 1],
                                op0=ALU.subtract, op1=ALU.mult)
        hs.append(h)
    return hs


def _lnt(tc, ctx, nc, src_tiles, hT, ident, psum_tr, stat_pool, h_pool,
         name="", copy_eng=None):
    """LayerNorm (token-major f32 src tiles [128,512]) -> bf16, then PE
    transpose into feature-major hT tiles: hT[cc][:, t*128:(t+1)*128].
    src_tiles: list of SBUF tiles [128, 512] f32."""
    copy = nc.scalar.copy if copy_eng == "scalar" else nc.vector.tensor_copy
    nt = len(src_tiles)
    hs = []
    for tg in range((nt + 3) // 4):
        grp = list(range(tg * 4, min(tg * 4 + 4, nt)))
        hs.extend(_ln_group(nc, stat_pool, h_pool,
                            [src_tiles[t] for t in grp], name=name))
    for tg in range((nt + 3) // 4):
        grp = list(range(tg * 4, min(tg * 4 + 4, nt)))
        for cc in range(4):
            ps = psum_tr.tile([128, 512], BF16, name=f"pst{name}", tag="pst")
            for k, t in enumerate(grp):
                nc.tensor.transpose(ps[:, k * 128:(k + 1) * 128],
                                    hs[t][:, cc * 128:(cc + 1) * 128],
                                    ident[:])
            w = len(grp) * 128
            copy(hT[cc][:, tg * 512:tg * 512 + w], ps[:, :w])


def build_nc(debug_taps=False):
    nc = bacc.Bacc("TRN2", target_bir_lowering=False, debug=False,
                   enable_asserts=False, num_devices=8)
    dbg = {}

    def tap(name, tiles, rows=128):
        """DMA a list of SBUF tiles out to a debug DRAM tensor."""
        if not debug_taps:
            return
        ap0 = tiles[0][:]
        cols = ap0.shape[-1]
        dt = ap0.dtype
        t = nc.dram_tensor(f"dbg_{name}", [len(tiles) * rows, cols], dt,
                           kind="ExternalOutput").ap()
        for i, ti in enumerate(tiles):
            nc.sync.dma_start(t[i * rows:(i + 1) * rows, :], ti[:rows, :])
        dbg[name] = t
    dt_in = {}

    def din(name, shape, dt):
        dt_in[name] = nc.dram_tensor(name, shape, dt, kind="ExternalInput").ap()
        return dt_in[name]

    x_own = din("x_own", [NC, D], F32)
    ctxT = din("ctxT", [CD, M], BF16)
    wq1 = din("wq1", [D, D], BF16)
    wk1 = din("wk1", [D, D], BF16)
    wv1 = din("wv1", [D, D], BF16)
    wo1 = din("wo1", [D, D], BF16)
    wq2 = din("wq2", [D, D], BF16)
    wk2 = din("wk2", [CD, D], BF16)
    wv2 = din("wv2", [CD, D], BF16)
    wo2 = din("wo2", [D, D], BF16)
    wfi = din("wfi", [D, 2 * FF], F8)
    wfo = din("wfo", [FF, D], F8)
    ident_d = din("ident", [128, 128], BF16)
    y = nc.dram_tensor("y", [NC, D], F32, kind="ExternalOutput").ap()

    xo_t = x_own.rearrange("(t p) d -> t p d", p=128)

    # DRAM staging for the K/V all-gather across the 4 cores of each batch
    k_loc = nc.dram_tensor("k_loc", [4 * 128, NC], BF16).ap()
    k_gath = nc.dram_tensor("k_gath", [16 * 128, NC], BF16).ap()
    v_loc = nc.dram_tensor("v_loc", [4 * 128, 2 * 528], F8).ap()
    v_gath = nc.dram_tensor("v_gath", [16 * 128, 2 * 528], F8).ap()
    RG = [[0, 1, 2, 3], [4, 5, 6, 7]]

    with tile.TileContext(nc) as tc, ExitStack() as top:
        const = top.enter_context(tc.tile_pool(name="const", bufs=1))
        ident = const.tile([128, 128], BF16)
        nc.sync.dma_start(ident[:], ident_d[:])

        y1_pool = top.enter_context(tc.tile_pool(name="y1", bufs=1))
        y1 = [y1_pool.tile([128, D], F32, name=f"y1_{i}", tag=f"y1_{i}")
              for i in range(NCT)]
        wq2p = top.enter_context(tc.tile_pool(name="wq2p", bufs=1))
        wq2_t = [wq2p.tile([128, D], BF16, name=f"wq2_{i}", tag=f"wq2_{i}")
                 for i in range(4)]
        for i in range(4):
            nc.sync.dma_start(wq2_t[i][:], wq2[i * 128:(i + 1) * 128, :])
        h2T_pool = top.enter_context(tc.tile_pool(name="h2T", bufs=1))
        h2T = [h2T_pool.tile([128, NC], BF16, name=f"h2T{i}", tag=f"h2T{i}")
               for i in range(4)]
        q2T_pool = top.enter_context(tc.tile_pool(name="q2T", bufs=1))
        q2Tz = [[q2T_pool.tile([128, NC], BF16, name=f"q2T{hh}_{i}",
                               tag=f"q2T{hh}_{i}") for i in range(4)]
                for hh in range(2)]
        for hh in range(2):
            for i in range(4):
                nc.gpsimd.memset(
                    q2Tz[hh][i][64 * (1 - hh):64 * (1 - hh) + 64, :], 0.0)

        # ---------------- Phase A: self-attention ----------------
        with ExitStack() as pa:
            w1 = pa.enter_context(tc.tile_pool(name="w1", bufs=1))
            wq1_t = [w1.tile([128, D], BF16, name=f"wq1_{i}", tag=f"wq1_{i}") for i in range(4)]
            wk1_t = [w1.tile([128, D], BF16, name=f"wk1_{i}", tag=f"wk1_{i}") for i in range(4)]
            wv1_t = [w1.tile([128, D], BF16, name=f"wv1_{i}", tag=f"wv1_{i}") for i in range(4)]
            wo1_t = [w1.tile([128, D], BF16, name=f"wo1_{i}", tag=f"wo1_{i}") for i in range(4)]
            kT_pool = pa.enter_context(tc.tile_pool(name="kT", bufs=1))
            kT = [kT_pool.tile([128, N], BF16, name=f"kT{i}", tag=f"kT{i}")
                  for i in range(4)]
            # fp8 V tiles with key-chunk pairs interleaved for DoubleRow:
            # va8[jcp][:, j*528 + h*66 + e] = v[chunk 2*jcp+j][key, h, e]
            # (e==64 is the ones column for the softmax denominator row).
            va_pool = pa.enter_context(tc.tile_pool(name="va", bufs=1))
            va8 = [va_pool.tile([128, 2 * 8 * 66], F8, name=f"va{i}",
                                tag=f"va{i}") for i in range(NT // 2)]
            # zero-padded per-head-half q tiles: qTz[hh] has head-half hh
            # rows live and the other 64 rows zero, so sim matmuls use the
            # full 128-row array (keeps the HAM clock gate open — half-row
            # tile_position matmuls run at 1.2 GHz)
            qT_pool = pa.enter_context(tc.tile_pool(name="qT", bufs=1))
            qTz = [[qT_pool.tile([128, NC], BF16, name=f"qT{hh}_{i}",
                                 tag=f"qT{hh}_{i}") for i in range(4)]
                   for hh in range(2)]
            for hh in range(2):
                for i in range(4):
                    nc.gpsimd.memset(
                        qTz[hh][i][64 * (1 - hh):64 * (1 - hh) + 64, :], 0.0)
            xo_pool = pa.enter_context(tc.tile_pool(name="xo", bufs=1))
            xo = [xo_pool.tile([128, D], F32, name=f"xo{i}", tag=f"xo{i}")
                  for i in range(NCT)]
            for i in range(NCT):
                nc.sync.dma_start(xo[i][:], xo_t[i])
            oT_pool = pa.enter_context(tc.tile_pool(name="oT", bufs=1))
            oT = [[oT_pool.tile([128, 512], BF16, name=f"oT{d}_{i}",
                                tag=f"oT{d}_{i}") for i in range(2)]
                  for d in range(4)]

            # --- projections: LN1 + q/k/v for the OWN 1024-token chunk
            # only; K and V(fp8 pair layout) are then all-gathered across
            # the 4 cores of this batch via DRAM collectives ---
            with ExitStack() as pp:
                hoT_pool = pp.enter_context(tc.tile_pool(name="hoT", bufs=1))
                hoT = [hoT_pool.tile([128, NC], BF16, name=f"hoT{i}",
                                     tag=f"hoT{i}") for i in range(4)]
                kown_pool = pp.enter_context(tc.tile_pool(name="kown", bufs=1))
                k_own = [kown_pool.tile([128, NC], BF16, name=f"ko{i}",
                                        tag=f"ko{i}") for i in range(4)]
                psum_tr = pp.enter_context(
                    tc.tile_pool(name="ptr", bufs=2, space="PSUM"))
                stat_pool = pp.enter_context(tc.tile_pool(name="stat", bufs=4))
                h_pool = pp.enter_context(tc.tile_pool(name="hp", bufs=6))

                # weight DMAs issued after x so they don't delay the LN path
                for i in range(4):
                    nc.sync.dma_start(wq1_t[i][:], wq1[i * 128:(i + 1) * 128, :])
                    nc.sync.dma_start(wk1_t[i][:], wk1[i * 128:(i + 1) * 128, :])
                    nc.sync.dma_start(wv1_t[i][:], wv1[i * 128:(i + 1) * 128, :])
                    nc.sync.dma_start(wo1_t[i][:], wo1[i * 128:(i + 1) * 128, :])
                psum_pj = pp.enter_context(
                    tc.tile_pool(name="ppj", bufs=4, space="PSUM"))
                _lnt(tc, pp, nc, xo, hoT, ident, psum_tr, stat_pool,
                     h_pool, name="lo", copy_eng="scalar")
                # k/v/q projections for the own chunk from hoT
                for jg in range(NC // 512):
                    for dc in range(4):
                        ps = psum_pj.tile([128, 512], F32, name="pk", tag="pj")
                        for cc in range(4):
                            nc.tensor.matmul(
                                ps[:],
                                lhsT=wk1_t[cc][:, dc * 128:(dc + 1) * 128],
                                rhs=hoT[cc][:, jg * 512:(jg + 1) * 512],
                                start=(cc == 0), stop=(cc == 3))
                        nc.scalar.copy(
                            k_own[dc][:, jg * 512:(jg + 1) * 512], ps[:])
                va_own_pool = pp.enter_context(
                    tc.tile_pool(name="vaown", bufs=1))
                va_own = [va_own_pool.tile([128, 2 * 528], F8,
                                           name=f"vo{i}", tag=f"vo{i}")
                          for i in range(4)]
                for jt in range(NCT):
                    ps = psum_pj.tile([128, 512], F32, name="pv", tag="pj")
                    for cc in range(4):
                        nc.tensor.matmul(
                            ps[:],
                            lhsT=hoT[cc][:, jt * 128:(jt + 1) * 128],
                            rhs=wv1_t[cc][:],
                            start=(cc == 0), stop=(cc == 3))
                    vbase = (jt % 2) * 528
                    va_r = va_own[jt // 2][:, vbase:vbase + 528].rearrange(
                        "p (h e) -> p h e", e=66)
                    nc.vector.tensor_copy(
                        va_r[:, :, 0:DH],
                        ps[:].rearrange("p (h e) -> p h e", e=DH))
                    nc.gpsimd.memset(va_r[:, :, DH:DH + 1], 1.0)
                for jg in range(NC // 512):
                    for dc in range(4):
                        ps = psum_pj.tile([128, 512], F32, name="pq", tag="pj")
                        for cc in range(4):
                            nc.tensor.matmul(
                                ps[:],
                                lhsT=wq1_t[cc][:, dc * 128:(dc + 1) * 128],
                                rhs=hoT[cc][:, jg * 512:(jg + 1) * 512],
                                start=(cc == 0), stop=(cc == 3))
                        nc.scalar.copy(
                            qTz[0][dc][0:64, jg * 512:(jg + 1) * 512],
                            ps[0:64, :])
                        nc.scalar.copy(
                            qTz[1][dc][64:128, jg * 512:(jg + 1) * 512],
                            ps[64:128, :])
                # ship own K/V to DRAM, all-gather, pull the full-sequence
                # K/V back into SBUF
                for dc in range(4):
                    nc.sync.dma_start(
                        k_loc[dc * 128:(dc + 1) * 128, :], k_own[dc][:])
                for jj in range(4):
                    nc.sync.dma_start(
                        v_loc[jj * 128:(jj + 1) * 128, :], va_own[jj][:])
                nc.gpsimd.collective_compute(
                    "AllGather", mybir.AluOpType.bypass,
                    replica_groups=RG, ins=[k_loc[:]], outs=[k_gath[:]])
                nc.gpsimd.collective_compute(
                    "AllGather", mybir.AluOpType.bypass,
                    replica_groups=RG, ins=[v_loc[:]], outs=[v_gath[:]])
                for g in range(4):
                    for dc in range(4):
                        nc.sync.dma_start(
                            kT[dc][:, g * NC:(g + 1) * NC],
                            k_gath[(g * 4 + dc) * 128:
                                   (g * 4 + dc + 1) * 128, :])
                    for jj in range(4):
                        nc.sync.dma_start(
                            va8[g * 4 + jj][:],
                            v_gath[(g * 4 + jj) * 128:
                                   (g * 4 + jj + 1) * 128, :])
                tap("hoT", hoT)
            # --- attention main loop: ic-major with paired heads.
            # PSUM pools are scoped per ic chunk so that finish_ic (to_out1,
            # LN2, h2T, q2T) gets banks and overlaps the next chunk's
            # attention via Tile's dependency tracking. ---
            o_pool = pa.enter_context(tc.tile_pool(name="o_t", bufs=1))
            o_t = [[o_pool.tile([128, 4 * DH], BF16, name=f"o{h}_{i}",
                                tag=f"o{h}_{i}") for i in range(2)]
                   for h in range(H)]
            sm_pool = pa.enter_context(tc.tile_pool(name="sm", bufs=4))
            st2 = pa.enter_context(tc.tile_pool(name="st2", bufs=4))
            h2_pool = pa.enter_context(tc.tile_pool(name="hp2", bufs=6))
            # flat fp8 probability buffers, one per head of the current
            # head-pair: chunk-major so DoubleRow pairs are adjacent slices
            p8_pool = pa.enter_context(tc.tile_pool(name="p8", bufs=1))
            p8 = [p8_pool.tile([128, NT * 512], F8, name=f"p8_{hh}",
                               tag=f"p8_{hh}") for hh in range(2)]
            groups = [list(range(g * 3, min(g * 3 + 3, NT)))
                      for g in range((NT + 2) // 3)]

            def attn1_ic(ic, psS, psA):
                for hp in range(4):
                    oacc = [psA.tile([65, 512], F32, name=f"oa{hh}",
                                     tag="oa") for hh in range(2)]
                    nxt = [0, 0]
                    for grp in groups:
                        w = len(grp) * 512
                        pss = [psS.tile([128, 1536], F32, name="sim",
                                        tag="sim") for hh in range(2)]
                        # full 128-row matmuls (the other head-half of the
                        # moving operand is zero) so the PE stays HAM-warm
                        for k, jc in enumerate(grp):
                            for hh in range(2):
                                nc.tensor.matmul(
                                    pss[hh][:, k * 512:(k + 1) * 512],
                                    lhsT=kT[hp][:, jc * 128:(jc + 1) * 128],
                                    rhs=qTz[hh][hp][:,
                                                    ic * 512:(ic + 1) * 512],
                                    start=True, stop=True)
                        for hh in range(2):
                            nc.scalar.activation(
                                p8[hh][:, grp[0] * 512:grp[0] * 512 + w],
                                pss[hh][:, :w], AF.Exp)
                        # attn@v: fp8 DoubleRow over completed chunk pairs
                        for hh in range(2):
                            h = 2 * hp + hh
                            while 2 * nxt[hh] + 1 <= grp[-1]:
                                jcp = nxt[hh]
                                nxt[hh] += 1
                                nc.tensor.matmul(
                                    oacc[hh][:],
                                    lhsT=va8[jcp].rearrange(
                                        "p (two he) -> p two he", two=2)[
                                        :, :, h * 66:h * 66 + 65],
                                    rhs=p8[hh][:, jcp * 1024:
                                               (jcp + 1) * 1024].rearrange(
                                        "p (two n) -> p two n", two=2),
                                    perf_mode=DR,
                                    start=(jcp == 0),
                                    stop=(jcp == NT // 2 - 1),
                                    skip_group_check=True)
                    for hh in range(2):
                        h = 2 * hp + hh
                        oc = sm_pool.tile([65, 512], BF16, name="oc",
                                          tag="oc")
                        nc.vector.tensor_copy(oc[:], oacc[hh][:])
                        pst = psA.tile([128, 264], BF16, name="pstt",
                                       tag="oa")
                        for m in range(4):
                            nc.tensor.transpose(
                                pst[:, m * 66:m * 66 + 65],
                                oc[:, m * 128:(m + 1) * 128],
                                ident[0:65, 0:65])
                        recip = sm_pool.tile([128, 4], F32, name="rc",
                                             tag="rc")
                        nc.vector.reciprocal(
                            recip[:],
                            pst[:, 0:264].rearrange(
                                "p (k e) -> p k e", e=66)[:, :, 64:65])
                        for m in range(4):
                            nc.vector.tensor_scalar(
                                o_t[h][ic][:, m * 64:(m + 1) * 64],
                                pst[:, m * 66:m * 66 + 64],
                                recip[:, m:m + 1], None, op0=ALU.mult)

            def finish_ic(ic, pfin):
                psU = pfin.enter_context(
                    tc.tile_pool(name=f"psU{ic}", bufs=2, space="PSUM"))
                for h in range(H):
                    dc, base = h // 2, 64 * (h % 2)
                    ps = psU.tile([64, 512], BF16, name="psO", tag="u")
                    for m in range(4):
                        nc.tensor.transpose(
                            ps[:, m * 128:(m + 1) * 128],
                            o_t[h][ic][:, m * 64:(m + 1) * 64], ident[:])
                    nc.vector.tensor_copy(
                        oT[dc][ic][base:base + 64, :], ps[:])
                for m in range(4):
                    it = ic * 4 + m
                    ps = psU.tile([128, 512], F32, name="pu", tag="u")
                    for dc in range(4):
                        nc.tensor.matmul(
                            ps[:],
                            lhsT=oT[dc][ic][:, m * 128:(m + 1) * 128],
                            rhs=wo1_t[dc][:],
                            start=(dc == 0), stop=(dc == 3))
                    nc.vector.tensor_add(y1[it][:], ps[:], xo[it][:])
                h2s = _ln_group(nc, st2, h2_pool,
                                [y1[ic * 4 + m] for m in range(4)], name="2")
                for cc in range(4):
                    ps = psU.tile([128, 512], BF16, name="ph2", tag="u")
                    for k in range(4):
                        nc.tensor.transpose(
                            ps[:, k * 128:(k + 1) * 128],
                            h2s[k][:, cc * 128:(cc + 1) * 128], ident[:])
                    nc.vector.tensor_copy(
                        h2T[cc][:, ic * 512:(ic + 1) * 512], ps[:])
                for dc in range(4):
                    ps = psU.tile([128, 512], F32, name="pq2", tag="u")
                    for cc in range(4):
                        nc.tensor.matmul(
                            ps[:],
                            lhsT=wq2_t[cc][:, dc * 128:(dc + 1) * 128],
                            rhs=h2T[cc][:, ic * 512:(ic + 1) * 512],
                            start=(cc == 0), stop=(cc == 3))
                    nc.vector.tensor_copy(
                        q2Tz[0][dc][0:64, ic * 512:(ic + 1) * 512],
                        ps[0:64, :])
                    nc.vector.tensor_copy(
                        q2Tz[1][dc][64:128, ic * 512:(ic + 1) * 512],
                        ps[64:128, :])

            with ExitStack() as pat:
                psS = pat.enter_context(
                    tc.tile_pool(name="psS", bufs=2, space="PSUM"))
                psA = pat.enter_context(
                    tc.tile_pool(name="psA", bufs=2, space="PSUM"))
                attn1_ic(0, psS, psA)
                attn1_ic(1, psS, psA)
            with ExitStack() as pfin:
                finish_ic(0, pfin)
                finish_ic(1, pfin)

        tap("y1", y1)

        # ---------------- Phase B: cross-attention + FFN ----------------
        with ExitStack() as pb:
            # fp8 FFN weights in DoubleRow pair layout: the pair dim is the
            # contraction 128-chunk index (cc for wfi, gp for wfo)
            wf = pb.enter_context(tc.tile_pool(name="wf", bufs=1))
            wfi8 = [wf.tile([128, 2 * 2 * FF], F8, name=f"wfi{i}",
                            tag=f"wfi{i}") for i in range(2)]
            wfo8 = [wf.tile([128, 2 * D], F8, name=f"wfo{i}", tag=f"wfo{i}")
                    for i in range(FF // 256)]
            for i in range(2):
                for j in range(2):
                    nc.sync.dma_start(
                        wfi8[i][:, j * 2 * FF:(j + 1) * 2 * FF],
                        wfi[(2 * i + j) * 128:(2 * i + j + 1) * 128, :])
            for i in range(FF // 256):
                for j in range(2):
                    nc.sync.dma_start(
                        wfo8[i][:, j * D:(j + 1) * D],
                        wfo[(2 * i + j) * 128:(2 * i + j + 1) * 128, :])

            w2 = pb.enter_context(tc.tile_pool(name="w2", bufs=1))
            wk2_t = [w2.tile([128, D], BF16, name=f"wk2_{i}", tag=f"wk2_{i}") for i in range(6)]
            wv2_t = [w2.tile([128, D], BF16, name=f"wv2_{i}", tag=f"wv2_{i}") for i in range(6)]
            wo2_t = [w2.tile([128, D], BF16, name=f"wo2_{i}", tag=f"wo2_{i}") for i in range(4)]
            ctx_t = [w2.tile([128, M], BF16, name=f"ctx{i}", tag=f"ctx{i}") for i in range(6)]
            for i in range(4):
                nc.sync.dma_start(wo2_t[i][:], wo2[i * 128:(i + 1) * 128, :])
            for i in range(6):
                nc.sync.dma_start(wk2_t[i][:], wk2[i * 128:(i + 1) * 128, :])
                nc.sync.dma_start(wv2_t[i][:], wv2[i * 128:(i + 1) * 128, :])
                nc.sync.dma_start(ctx_t[i][:], ctxT[i * 128:(i + 1) * 128, :])

            y2_pool = pb.enter_context(tc.tile_pool(name="y2", bufs=1))
            y2 = [y2_pool.tile([128, D], F32, name=f"y2_{i}", tag=f"y2_{i}")
                  for i in range(NCT)]

            k2T_pool = pb.enter_context(tc.tile_pool(name="k2T", bufs=1))
            k2T = [k2T_pool.tile([128, M], BF16, name=f"k2T{i}", tag=f"k2T{i}")
                   for i in range(4)]
            va2_pool = pb.enter_context(tc.tile_pool(name="va2", bufs=1))
            va28 = va2_pool.tile([128, 2 * 8 * 66], F8, name="va28",
                                 tag="va28")
            o2T_pool = pb.enter_context(tc.tile_pool(name="o2T", bufs=1))
            o2T = [[o2T_pool.tile([128, 512], BF16, name=f"o2T{d}_{i}",
                                  tag=f"o2T{d}_{i}") for i in range(2)]
                   for d in range(4)]

            # --- projections for cross-attn (k2/v2 from context only;
            # h2T/q2T were produced during the attention overlap) ---
            with ExitStack() as pp2:
                psum_p2 = pp2.enter_context(
                    tc.tile_pool(name="pp2", bufs=4, space="PSUM"))
                for dc in range(4):
                    ps = psum_p2.tile([128, M], F32, name="pk2", tag="p2")
                    for cc in range(6):
                        nc.tensor.matmul(
                            ps[:],
                            lhsT=wk2_t[cc][:, dc * 128:(dc + 1) * 128],
                            rhs=ctx_t[cc][:],
                            start=(cc == 0), stop=(cc == 5))
                    nc.vector.tensor_copy(k2T[dc][:], ps[:])
                for jt in range(2):
                    ps = psum_p2.tile([128, 512], F32, name="pv2", tag="p2")
                    for cc in range(6):
                        nc.tensor.matmul(
                            ps[:],
                            lhsT=ctx_t[cc][:, jt * 128:(jt + 1) * 128],
                            rhs=wv2_t[cc][:],
                            start=(cc == 0), stop=(cc == 5))
                    vbase = jt * 528
                    va_r = va28[:, vbase:vbase + 528].rearrange(
                        "p (h e) -> p h e", e=66)
                    nc.vector.tensor_copy(
                        va_r[:, :, 0:DH],
                        ps[:].rearrange("p (h e) -> p h e", e=DH))
                    nc.gpsimd.memset(va_r[:, :, DH:DH + 1], 1.0)

            # --- cross-attention loop (2 key tiles) ---
            o2_pool = pb.enter_context(tc.tile_pool(name="o2_t", bufs=1))
            o2_t = [[o2_pool.tile([128, 4 * DH], BF16, name=f"o2{h}_{i}",
                                  tag=f"o2{h}_{i}") for i in range(2)]
                    for h in range(H)]
            with ExitStack() as pat2:
                psS2 = pat2.enter_context(
                    tc.tile_pool(name="psS2", bufs=2, space="PSUM"))
                psA2 = pat2.enter_context(
                    tc.tile_pool(name="psA2", bufs=2, space="PSUM"))
                pP2 = pat2.enter_context(tc.tile_pool(name="pP2", bufs=4))
                sm2 = pat2.enter_context(tc.tile_pool(name="sm2", bufs=4))
                for hp in range(4):
                    for ic in range(2):
                        oacc = [psA2.tile([65, 512], F32, name=f"o2a{hh}",
                                          tag="o2a") for hh in range(2)]
                        pss = [psS2.tile([128, 1024], F32, name="sim2",
                                         tag="sim2") for hh in range(2)]
                        for jc in range(2):
                            for hh in range(2):
                                nc.tensor.matmul(
                                    pss[hh][:, jc * 512:(jc + 1) * 512],
                                    lhsT=k2T[hp][:, jc * 128:(jc + 1) * 128],
                                    rhs=q2Tz[hh][hp][:,
                                                     ic * 512:
                                                     (ic + 1) * 512],
                                    start=True, stop=True)
                        for hh in range(2):
                            h = 2 * hp + hh
                            p = pP2.tile([128, 1024], F8, name="p2", tag="p2")
                            nc.scalar.activation(p[:], pss[hh][:], AF.Exp)
                            nc.tensor.matmul(
                                oacc[hh][:],
                                lhsT=va28.rearrange(
                                    "p (two he) -> p two he", two=2)[
                                    :, :, h * 66:h * 66 + 65],
                                rhs=p[:].rearrange(
                                    "p (two n) -> p two n", two=2),
                                perf_mode=DR, start=True, stop=True)
                        for hh in range(2):
                            h = 2 * hp + hh
                            oc = sm2.tile([65, 512], BF16, name="oc2",
                                          tag="oc2")
                            nc.vector.tensor_copy(oc[:], oacc[hh][:])
                            pst = psA2.tile([128, 264], BF16, name="pstt2",
                                            tag="o2a")
                            for m in range(4):
                                nc.tensor.transpose(
                                    pst[:, m * 66:m * 66 + 65],
                                    oc[:, m * 128:(m + 1) * 128],
                                    ident[0:65, 0:65])
                            recip = sm2.tile([128, 4], F32, name="rc2",
                                             tag="rc2")
                            nc.vector.reciprocal(
                                recip[:],
                                pst[:, 0:264].rearrange(
                                    "p (k e) -> p k e", e=66)[:, :, 64:65])
                            for m in range(4):
                                nc.vector.tensor_scalar(
                                    o2_t[h][ic][:, m * 64:(m + 1) * 64],
                                    pst[:, m * 66:m * 66 + 64],
                                    recip[:, m:m + 1], None, op0=ALU.mult)

            with ExitStack() as pot2:
                psO2 = pot2.enter_context(
                    tc.tile_pool(name="psO2", bufs=4, space="PSUM"))
                for h in range(H):
                    dc, base = h // 2, 64 * (h % 2)
                    for ic in range(2):
                        ps = psO2.tile([64, 512], BF16, name="pso2", tag="pso2")
                        for m in range(4):
                            nc.tensor.transpose(
                                ps[:, m * 128:(m + 1) * 128],
                                o2_t[h][ic][:, m * 64:(m + 1) * 64],
                                ident[:])
                        nc.vector.tensor_copy(
                            o2T[dc][ic][base:base + 64, :], ps[:])

            with ExitStack() as pto2:
                psU2 = pto2.enter_context(
                    tc.tile_pool(name="psU2", bufs=2, space="PSUM"))
                for ic in range(2):
                    for m in range(4):
                        it = ic * 4 + m
                        ps = psU2.tile([128, 512], F32, name="pu2", tag="pu2")
                        for dc in range(4):
                            nc.tensor.matmul(
                                ps[:],
                                lhsT=o2T[dc][ic][:, m * 128:(m + 1) * 128],
                                rhs=wo2_t[dc][:],
                                start=(dc == 0), stop=(dc == 3))
                        nc.vector.tensor_add(y2[it][:], ps[:], y1[it][:])

            tap("k2T", k2T)

            tap("o2_t", [o2_t[h][i] for h in range(H) for i in range(2)])
            tap("y2", y2)

            # ---------------- FFN (GEGLU, fp8 DoubleRow) ----------------
            ffT_pool = pb.enter_context(tc.tile_pool(name="ffT", bufs=1))
            ffT8 = [ffT_pool.tile([128, 2 * NC], F8, name=f"ffT{i}",
                                  tag=f"ffT{i}") for i in range(FF // 256)]
            with ExitStack() as pf:
                h3T_pool = pf.enter_context(tc.tile_pool(name="h3T", bufs=1))
                h3T8 = [h3T_pool.tile([128, 2 * NC], F8, name=f"h3T{i}",
                                      tag=f"h3T{i}") for i in range(2)]
                h3T_v = [h3T8[cc // 2][:, (cc % 2) * NC:(cc % 2 + 1) * NC]
                         for cc in range(4)]
                psum_tr3 = pf.enter_context(
                    tc.tile_pool(name="ptr3", bufs=2, space="PSUM"))
                stat3 = pf.enter_context(tc.tile_pool(name="stat3", bufs=4))
                h3_pool = pf.enter_context(tc.tile_pool(name="hp3", bufs=6))
                _lnt(tc, pf, nc, y2, h3T_v, ident, psum_tr3, stat3, h3_pool,
                     name="l3", copy_eng="scalar")

                psum_g = pf.enter_context(
                    tc.tile_pool(name="pg", bufs=6, space="PSUM"))
                gl_pool = pf.enter_context(tc.tile_pool(name="gl", bufs=3))
                wfi8_r = [wfi8[i].rearrange("p (two n) -> p two n", two=2)
                          for i in range(2)]
                h3T8_r = [h3T8[i].rearrange("p (two n) -> p two n", two=2)
                          for i in range(2)]
                for gp in range(FF // 128):
                    psv = [psum_g.tile([128, 512], F32, name=f"psv{ic}",
                                       tag="pg") for ic in range(2)]
                    psg = [psum_g.tile([128, 512], F32, name=f"psg{ic}",
                                       tag="pg") for ic in range(2)]
                    for off, pst_ in ((0, psv), (FF, psg)):
                        for ccp in range(2):
                            for ic in range(2):
                                nc.tensor.matmul(
                                    pst_[ic][:],
                                    lhsT=wfi8_r[ccp][
                                        :, :,
                                        off + gp * 128:off + (gp + 1) * 128],
                                    rhs=h3T8_r[ccp][
                                        :, :, ic * 512:(ic + 1) * 512],
                                    perf_mode=DR,
                                    start=(ccp == 0), stop=(ccp == 1),
                                    skip_group_check=True)
                    for ic in range(2):
                        gl = gl_pool.tile([128, 512], BF16, name="glt",
                                          tag="gl")
                        nc.scalar.activation(gl[:], psg[ic][:], AF.Gelu)
                        nc.vector.tensor_mul(
                            ffT8[gp // 2][:, (gp % 2) * NC + ic * 512:
                                          (gp % 2) * NC + (ic + 1) * 512],
                            psv[ic][:], gl[:])

            # --- ff_out + residual -> DMA out ---
            with ExitStack() as pfo:
                psF = pfo.enter_context(
                    tc.tile_pool(name="psF", bufs=2, space="PSUM"))
                y3_pool = pfo.enter_context(tc.tile_pool(name="y3", bufs=3))
                ffT8_r = [t.rearrange("p (two n) -> p two n", two=2)
                          for t in ffT8]
                wfo8_r = [t.rearrange("p (two n) -> p two n", two=2)
                          for t in wfo8]
                for it in range(NCT):
                    ps = psF.tile([128, 512], F32, name="pf", tag="pf")
                    for gpp in range(FF // 256):
                        nc.tensor.matmul(
                            ps[:],
                            lhsT=ffT8_r[gpp][:, :, it * 128:(it + 1) * 128],
                            rhs=wfo8_r[gpp][:],
                            perf_mode=DR,
                            start=(gpp == 0), stop=(gpp == FF // 256 - 1),
                            skip_group_check=True)
                    y3 = y3_pool.tile([128, D], F32, name="y3t", tag="y3")
                    nc.vector.tensor_add(y3[:], ps[:], y2[it][:])
                    nc.sync.dma_start(y[it * 128:(it + 1) * 128, :], y3[:])

    nc.compile()
    return nc


_CACHE = {}


def get_nc(debug_taps=False):
    key = ("nc", debug_taps)
    if key not in _CACHE:
        _CACHE[key] = build_nc(debug_taps)
    return _CACHE[key]


def make_in_maps(x, context, q1_w, k1_w, v1_w, o1_w, o1_b,
                 q2_w, k2_w, v2_w, o2_w, o2_b,
                 ff_in_w, ff_in_b, ff_out_w, ff_out_b,
                 ln1_g, ln1_b, ln2_g, ln2_b, ln3_g, ln3_b):
    for b_ in (o1_b, o2_b, ff_in_b, ff_out_b, ln1_b, ln2_b, ln3_b):
        assert not np.any(np.asarray(b_)), "nonzero biases not supported"
    bf = ml_dtypes.bfloat16
    wq1 = (np.asarray(ln1_g)[:, None] * np.asarray(q1_w) * SCALE).astype(bf)
    wk1 = (np.asarray(ln1_g)[:, None] * np.asarray(k1_w)).astype(bf)
    wv1 = (np.asarray(ln1_g)[:, None] * np.asarray(v1_w)).astype(bf)
    wo1 = np.asarray(o1_w).astype(bf)
    wq2 = (np.asarray(ln2_g)[:, None] * np.asarray(q2_w) * SCALE).astype(bf)
    wk2 = np.asarray(k2_w).astype(bf)
    wv2 = np.asarray(v2_w).astype(bf)
    wo2 = np.asarray(o2_w).astype(bf)
    f8 = ml_dtypes.float8_e4m3
    wfi = (np.asarray(ln3_g)[:, None] * np.asarray(ff_in_w)).astype(f8)
    wfo = np.asarray(ff_out_w).astype(f8)
    ident = np.eye(128, dtype=bf)
    x = np.asarray(x, dtype=np.float32)
    ctxT = np.ascontiguousarray(
        np.asarray(context, dtype=np.float32).transpose(0, 2, 1)).astype(bf)

    in_maps = []
    for c in range(8):
        b_, ch = c // 4, c % 4
        in_maps.append({
            "x_own": np.ascontiguousarray(x[b_, ch * NC:(ch + 1) * NC]),
            "ctxT": np.ascontiguousarray(ctxT[b_]),
            "wq1": wq1, "wk1": wk1, "wv1": wv1, "wo1": wo1,
            "wq2": wq2, "wk2": wk2, "wv2": wv2, "wo2": wo2,
            "wfi": wfi, "wfo": wfo, "ident": ident,
        })
    return in_maps


def kernel(**inputs):
    nc = get_nc()
    in_maps = make_in_maps(**inputs)
    res = bass_utils.run_bass_kernel_spmd(nc, in_maps, core_ids=list(range(8)))
    out = np.empty((B, N, D), dtype=np.float32)
    for c in range(8):
        b_, ch = c // 4, c % 4
        out[b_, ch * NC:(ch + 1) * NC] = res.results[c]["y"]
    return out

